# revision 21
# baseline (speedup 1.0000x reference)
"""Trainium2 Bass kernel v4 for upsample_conv_2d — polyphase conv + split FIR.

The baseline folded the whole 4x4 FIR into the conv weights, turning the op
into four phase-specific 3x3 convs (36 dense 256x256 taps, ~590K PE columns).
This version balances the work across engines:

  1. Stage A (PE): the stride-2 3x3 conv_transpose composed with ONE
     horizontal 2-tap box filter ([1,1] of the FIR factorization
     [1,3,3,1] = [1,1]*[1,2,1]), as 12 polyphase taps (~202K PE columns,
     2.9x less tensor work than baseline). The box folds into the
     weights (heff = [w0, w0+w1, w1+w2, w2] polyphase), not into extra
     matmul streams. PSUM accumulates fp32; the Scalar engine evicts
     each PSUM group into an SBUF fp16 buffer z[131,130] with the two
     column phases interleaved (strided writes), adding bias/32.
  2. Stage B (DVE + Pool): the remaining FIR — vertical [1,3,3,1]
     (three 2-tap box passes) and horizontal [1,2,1] (two box passes) —
     as tensor_add ops (fp16 packed => 2x DVE perf mode), each pass
     row-split between the Vector and GpSimd engines (GpSimd runs
     tensor ops ~3x slower, so it gets ~24% of rows). Each co-block has
     its own (z, t) buffer pair, ping-ponged, so the two chains share
     nothing and overlap stage A of the other block freely.

The conv bias is pre-divided by 32 (the unnormalized mass of the
remaining FIR chain), added during eviction and preset on the top/bottom
ring rows, so the box chain amplifies it back to exactly +bias. The 1/16
FIR normalization is folded into host-side x and w scaling (0.25 each).
Output is staged packed [128, 128*128] fp16 and shipped in 2 DMAs of
2 MB per co-block (16 KB contiguous per partition).
"""

import json
import os as _os

import numpy as np

import concourse.bass as bass
import concourse.mybir as mybir
import concourse.tile as tile
from concourse.bass_utils import run_bass_kernel_spmd

# ---------------------------------------------------------------------------
# BIR post-pass: this walrus build rejects instructions carrying more than one
# sem wait (e.g. Tile's kernel-tail Drain gets 3). Hoist extras into
# standalone EventSemaphore instructions right before the owner.
# ---------------------------------------------------------------------------
_MAX_WAITS = 1


def _split_waits(j: dict) -> dict:
    for fn in j.get("functions", []):
        for blk in fn.get("blocks", []):
            insts = blk.get("instructions")
            if not insts:
                continue
            out = []
            for inst in insts:
                si = inst.get("sync_info") or {}
                waits = si.get("on_wait") or []
                if len(waits) > _MAX_WAITS:
                    for k, w in enumerate(waits[_MAX_WAITS:]):
                        out.append(
                            {
                                "debug": inst.get("debug", 0),
                                "engine": inst["engine"],
                                "ins": [],
                                "name": f"{inst['name']}-wsplit{k}",
                                "opcode": "EventSemaphore",
                                "outs": [],
                                "sync_info": {"on_update": [], "on_wait": [w]},
                            }
                        )
                    si["on_wait"] = waits[:_MAX_WAITS]
                out.append(inst)
            blk["instructions"] = out
    return j


_orig_to_json_bytes = bass.Bass.to_json_bytes


def _patched_to_json_bytes(self):
    return json.dumps(_split_waits(json.loads(_orig_to_json_bytes(self)))).encode()


bass.Bass.to_json_bytes = _patched_to_json_bytes

# ---------------------------------------------------------------------------
# Problem constants (hardcoded; kernel.py must be self-contained)
# ---------------------------------------------------------------------------
N, C, H, W = 8, 256, 64, 64
OH, OW = 2 * H, 2 * W
N_CORES = 8
F32 = mybir.dt.float32
F16 = mybir.dt.float16

# Fraction of each FIR pass's rows run on DVE (rest on GpSimd/Pool).
# Measured on HW: DVE ~0.61 ns/elem (2x fp16), Pool ~1.89 ns/elem
# (GPSIMD Add runs at ~0.42 of its 1.2GHz roofline).
FIR_DVE_FRAC = float(_os.environ.get("FIR_DVE_FRAC", "0.79"))
# scalar_tensor_tensor on GPSIMD does not lower in this compiler build
# (walrus rejects it); keep the faster-Pool path available but off.
POOL_STT = _os.environ.get("POOL_STT", "0") == "1"
ALU_MULT = mybir.AluOpType.mult
ALU_ADD = mybir.AluOpType.add

# Polyphase taps of (stride-2 conv_transpose, full padding) composed with a
# horizontal 2-tap box [1,1]:
#   z2[1 + 2I + pv, 2J + pc] = sum_taps wsum * xpad[I + dr, J + dc]
# where xpad has a 1-px zero halo and wsum sums the listed (p, q) entries of
# the 3x3 weight. Entries: (pv, pc, dr, dc, [(p, q), ...])
TAPS = []
for pv, vparts in ((0, ((0, 0), (1, 2))), (1, ((1, 1),))):
    # vparts: (dr, p) pairs for this vertical phase
    for pc in (0, 1):
        for dr, p in vparts:
            for dc in (0, 1):
                if pc == 0:
                    qs = [(p, 0), (p, 1)] if dc == 0 else [(p, 2)]
                else:
                    qs = [(p, 0)] if dc == 0 else [(p, 1), (p, 2)]
                TAPS.append((pv, pc, dr, dc, qs))

NT = len(TAPS)  # 12
PHASE_ROWS = {0: 65, 1: 64}


def _tap_weight_matrix(w: np.ndarray) -> np.ndarray:
    """[256,256,3,3] conv_transpose weight -> [128, NT*4*128] fp16 lhsT.

    Column block index = (t*2 + cib)*2 + cob; row = ci within ci-block.
    Scaled 0.25 (with x also scaled 0.25 => the 1/16 FIR normalization).
    """
    ws = w.astype(np.float64) * 0.25
    Wmat = np.zeros((128, NT * 4, 128), dtype=np.float16)
    for t, (_pv, _pc, _dr, _dc, qs) in enumerate(TAPS):
        eff = np.zeros((256, 256), dtype=np.float64)
        for p, q in qs:
            eff += ws[:, :, p, q]
        for cib in range(2):
            for cob in range(2):
                blk = eff[
                    cob * 128 : (cob + 1) * 128, cib * 128 : (cib + 1) * 128
                ]  # [co, ci]
                Wmat[:, (t * 2 + cib) * 2 + cob, :] = blk.T.astype(np.float16)
    return Wmat.reshape(128, -1)


def _prep_inputs(x, weight, bias):
    Wmat = _tap_weight_matrix(np.asarray(weight, dtype=np.float32))
    # bias/32: the remaining FIR chain mass (vertical 8 x horizontal 4)
    # multiplies the ring + eviction bias back to exactly +bias.
    b2 = np.ascontiguousarray(
        (np.asarray(bias, dtype=np.float32) / 32.0).reshape(2, 128)
    )
    xs = (
        np.pad(np.asarray(x, dtype=np.float32), ((0, 0), (0, 0), (1, 1), (1, 1)))
        * 0.25
    ).astype(np.float16)
    return xs, Wmat, b2


def build_nc(reps: int = 1, loop: bool = False) -> bass.Bass:
    nc = bass.Bass("TRN2", target_bir_lowering=False, debug=False)
    x_d = nc.dram_tensor("x", [C, H + 2, W + 2], F16, kind="ExternalInput").ap()
    w_d = nc.dram_tensor("w", [128, NT * 4 * 128], F16, kind="ExternalInput").ap()
    b_d = nc.dram_tensor("bias", [2, 128], F32, kind="ExternalInput").ap()
    out_d = nc.dram_tensor("out", [C, OH, OW], F16, kind="ExternalOutput").ap()

    xb = x_d.rearrange("(b p) h w -> b p h w", p=128)

    IDENT = mybir.ActivationFunctionType.Identity

    with tile.TileContext(nc) as tc:
        with (
            tc.tile_pool(name="const", bufs=1) as cpool,
            tc.tile_pool(name="zbuf", bufs=1) as zpool,
            tc.tile_pool(name="psum", bufs=4, space="PSUM") as ppool,
        ):
            wt = cpool.tile([128, NT * 4, 128], F16)
            nc.sync.dma_start(wt[:], w_d.rearrange("p (a b) -> p a b", b=128))
            bt = cpool.tile([128, 2], F32)
            nc.sync.dma_start(bt[:], b_d.rearrange("b p -> p b"))

            # x arrives zero-padded to 66x66 (and pre-scaled 0.25) from host
            xpad = [
                cpool.tile([128, 66, 66], F16, tag=f"xp{i}", name=f"xp{i}")
                for i in range(2)
            ]
            for cib in range(2):
                nc.sync.dma_start(xpad[cib][:], xb[cib])

            # z: stage-A output (131 rows x 130 cols: bias/32 ring rows 0 and
            # 130, interior rows 1..129 evicted per rep). t: FIR ping-pong
            # partner. One (z, t) pair per co-block so the two FIR chains
            # share nothing.
            zb = [
                zpool.tile([128, 131, 130], F16, tag=f"z{i}", name=f"z{i}")
                for i in range(2)
            ]
            tb = [
                zpool.tile([128, 131, 130], F16, tag=f"t{i}", name=f"t{i}")
                for i in range(2)
            ]
            for i in range(2):
                nc.vector.memset(zb[i][:], 0.0)

            def body():
                for cob in range(2):
                    z = zb[cob]
                    t = tb[cob]
                    bcol = bt[:, cob : cob + 1]
                    zv = z[:, 1:131, :].rearrange(
                        "p (i a) (j b) -> p i a j b", a=2, b=2
                    )
                    # Re-init the bias/32 ring rows (the FIR chain reuses z
                    # as scratch and clobbers row 0; row 130 kept for
                    # symmetry/robustness).
                    for ring in (z[:, 0:1, :], z[:, 130:131, :]):
                        nc.scalar.activation(ring, ring, IDENT, bias=bcol, scale=0.0)
                    # ---- Stage A: polyphase matmuls + PSUM eviction ----
                    for pv in (0, 1):
                        for pc in (0, 1):
                            accums = [
                                (dr, dc, cib, (ti * 2 + cib) * 2 + cob)
                                for ti, (tpv, tpc, dr, dc, _qs) in enumerate(TAPS)
                                if tpv == pv and tpc == pc
                                for cib in range(2)
                            ]
                            rows = PHASE_ROWS[pv]
                            na = len(accums)
                            R = 7
                            i0 = 0
                            while i0 < rows:
                                g_rows = min(2 * R, rows - i0)
                                chunks = []
                                r0 = i0
                                while r0 < i0 + g_rows:
                                    cr = min(R, i0 + g_rows - r0)
                                    chunks.append((r0, cr))
                                    r0 += cr
                                P = ppool.tile([128, 2, 512], F32, tag="mm", name="mm")
                                for ai, (dr, dc, cib, widx) in enumerate(accums):
                                    lhsT = wt[:, widx, :]
                                    for ci_, (cr0, crn) in enumerate(chunks):
                                        rhs = xpad[cib][
                                            :, cr0 + dr : cr0 + dr + crn, dc : dc + 65
                                        ]
                                        outp = P[:, ci_, 0 : crn * 65].rearrange(
                                            "p (r w) -> p r w", w=65
                                        )
                                        nc.tensor.matmul(
                                            outp,
                                            lhsT,
                                            rhs,
                                            start=(ai == 0),
                                            stop=(ai == na - 1),
                                        )
                                # evict: uniform leading chunks in one op
                                nfull = 0
                                for _cr0, crn in chunks:
                                    if crn == R:
                                        nfull += 1
                                    else:
                                        break
                                if nfull:
                                    src = P[:, 0:nfull, 0 : R * 65].rearrange(
                                        "p c (r w) -> p c r w", w=65
                                    )
                                    dst = zv[
                                        :, i0 : i0 + nfull * R, pv, 0:65, pc
                                    ].rearrange("p (c r) j -> p c r j", r=R)
                                    nc.scalar.activation(dst, src, IDENT, bias=bcol)
                                for ci_ in range(nfull, len(chunks)):
                                    cr0, crn = chunks[ci_]
                                    src = P[:, ci_, 0 : crn * 65].rearrange(
                                        "p (r w) -> p r w", w=65
                                    )
                                    dst = zv[:, cr0 : cr0 + crn, pv, 0:65, pc]
                                    nc.scalar.activation(dst, src, IDENT, bias=bcol)
                                i0 += g_rows

                    # ---- Stage B: remaining FIR (vertical [1,3,3,1] +
                    # horizontal [1,2,1]) as 5 box passes, each row-split
                    # between DVE and Pool, ping-ponging z <-> t ----
                    def split(n):
                        nd = int(round(n * FIR_DVE_FRAC))
                        return ((nc.vector, 0, nd), (nc.gpsimd, nd, n))

                    tf = t[:].rearrange("p a b -> p (a b)")
                    ov = tf[:, 0:16384].rearrange("p (h w) -> p h w", w=128)
                    # (out, in, out_rows, vshift?, cols_out)
                    chain = [
                        (t, z, 130, True, 130),
                        (z, t, 129, True, 130),
                        (t, z, 128, True, 130),
                        (z, t, 128, False, 129),
                        (ov, z, 128, False, 128),
                    ]
                    for dst, srct, nrows, vert, co_ in chain:
                        for eng, r0, r1 in split(nrows):
                            if r0 == r1:
                                continue
                            if dst is ov:
                                d = ov[:, r0:r1, :]
                            else:
                                d = dst[:, r0:r1, 0:co_]
                            if vert:
                                a = srct[:, r0:r1, 0:co_]
                                b_ = srct[:, r0 + 1 : r1 + 1, 0:co_]
                            else:
                                a = srct[:, r0:r1, 0:co_]
                                b_ = srct[:, r0:r1, 1 : co_ + 1]
                            if eng is nc.gpsimd and POOL_STT:
                                # GPSIMD runs TensorScalarPtr at 0.60 roofline
                                # efficiency vs TensorTensor-Add's 0.42, so a
                                # box add as (a*1)+b is ~1.4x faster on Pool.
                                eng.scalar_tensor_tensor(
                                    d, a, 1.0, b_, ALU_MULT, ALU_ADD
                                )
                            else:
                                eng.tensor_add(d, a, b_)
                    for half in range(2):
                        dst = out_d[
                            cob * 128 : (cob + 1) * 128,
                            half * 64 : (half + 1) * 64,
                            :,
                        ].rearrange("c h w -> c (h w)")
                        nc.sync.dma_start(dst, tf[:, half * 8192 : (half + 1) * 8192])

            if loop:
                with tc.For_i(0, reps):
                    body()
            else:
                for _rep in range(reps):
                    body()
    return nc


_CACHED_NC = {}


def _get_nc(reps: int = 1, loop: bool = False) -> bass.Bass:
    key = (reps, loop)
    if key not in _CACHED_NC:
        _CACHED_NC[key] = build_nc(reps, loop)
    return _CACHED_NC[key]


def _run(x, weight, bias, reps: int = 1, loop: bool = False):
    xs, Wmat, b2 = _prep_inputs(x, weight, bias)
    nc = _get_nc(reps, loop)
    in_maps = [{"x": xs[i], "w": Wmat, "bias": b2} for i in range(N_CORES)]
    res = run_bass_kernel_spmd(nc, in_maps, list(range(N_CORES)))
    return np.stack(
        [res.results[i]["out"].astype(np.float32) for i in range(N_CORES)]
    )


def kernel(x, weight, bias):
    return _run(x, weight, bias, reps=1)
